# revision 14
# baseline (speedup 1.0000x reference)
"""Multi-head attention (B=2, N=2048, C=1024, H=16, D=64) on 8 TRN2 NeuronCores.

Sharding: tensor-parallel over heads. Core i owns heads (2i, 2i+1):
  - qkv weight columns for those heads (Q^T/K^T/V^T computed on device),
  - attention for 4 (batch, head) instances,
  - partial projection y_i = O_i @ W_proj[:, cols_i].T  (row-parallel proj).
Host gathers: y = sum_i y_i + b_proj.

Per-core pipeline (per batch):
  qkv:   Q^T,K^T [128(d,2 heads),2048] and V^T -> PE-transpose -> V_aug [n,130]
         (V columns + a ones column per head, so PV also yields softmax row-sums)
  attn:  per 512-query tile, loop over 16 key tiles:
         S^T[k,h,q] = K^T.T @ Q^T (bf16 in, f32 PSUM, the two heads run as
         packed row-group tiles), exp on ScalarE -> bf16,
         O~aug^T[65,q] += V_aug.T @ P~ (PSUM accumulation, row 64 = sum exp)
  norm:  recip(row 64) -> DMA partition-broadcast -> DVE mult
  proj:  y[q,o] = sum_h O_norm^T_h.T @ WpT_h (PSUM accumulation over heads)

Batch 1's qkv work is emitted interleaved with batch 0's attention so the
TensorEngine stays dense (HAM stays at full clock) while ScalarE runs exp.
Matmul operands are bf16; softmax statistics, PSUM accumulation and the
final output stay float32.
"""
import sys
import types

import numpy as np

B = 2
N = 2048
C = 1024
H = 16
D = 64
SCALE = D ** -0.5
NCORES = 8
HPC = H // NCORES  # heads per core = 2
BN = B * N


def _install_ntff_shim():
    """The image's antenv lacks axon_hooks; provide it so trace=True works."""
    if "antenv.axon_hooks" in sys.modules:
        return
    mod = types.ModuleType("antenv.axon_hooks")
    mod._HOOK = None
    mod.set_axon_ntff_profile_hook = lambda h: setattr(mod, "_HOOK", h)
    mod.get_axon_ntff_profile_hook = lambda: mod._HOOK
    sys.modules["antenv.axon_hooks"] = mod
    if "/root/.axon_site" not in sys.path:
        sys.path.insert(0, "/root/.axon_site")
    try:
        from trn_agent_boot.trn_boot import _ntff_profile_via_ctypes

        mod.set_axon_ntff_profile_hook(
            _ntff_profile_via_ctypes("/opt/axon/libaxon_pjrt.so")
        )
    except Exception:
        pass


_install_ntff_shim()

import ml_dtypes  # noqa: E402

import concourse.bass as bass  # noqa: E402
import concourse.tile as tile  # noqa: E402
from concourse import bacc, mybir  # noqa: E402
from concourse.bass_utils import run_bass_kernel_spmd  # noqa: E402
from concourse.masks import make_identity  # noqa: E402

F32 = mybir.dt.float32
BF16 = mybir.dt.bfloat16
EXP = mybir.ActivationFunctionType.Exp
BFNP = ml_dtypes.bfloat16

NT = N // 512          # 512-token tiles per batch (4)
NKT = N // 128         # 128-token key tiles per batch (16)
CO = C // 128          # contraction chunks (8)

_NC_CACHE = {}


def build_nc():
    nc = bacc.Bacc(None, target_bir_lowering=False)

    xT_ext = nc.declare_dram_parameter("xT", [C, BN], BF16, isOutput=False)
    wqT_ext = nc.declare_dram_parameter("wqT", [C, 128], BF16, isOutput=False)
    wkT_ext = nc.declare_dram_parameter("wkT", [C, 128], BF16, isOutput=False)
    wvT_ext = nc.declare_dram_parameter("wvT", [C, 128], BF16, isOutput=False)
    wpT_ext = nc.declare_dram_parameter("wpT", [HPC, D, C], BF16, isOutput=False)
    out_ext = nc.declare_dram_parameter("out", [BN, C], F32, isOutput=True)

    with tile.TileContext(nc) as tc:
        with (
            tc.tile_pool(name="consts", bufs=1) as consts,
            tc.tile_pool(name="weights", bufs=1) as weights,
            tc.tile_pool(name="xpool", bufs=8 * NT) as xpool,
            tc.tile_pool(name="qkvp", bufs=2 * NT) as qkvp,
            tc.tile_pool(name="work", bufs=8) as work,
            tc.tile_pool(name="small", bufs=3) as small,
            tc.tile_pool(name="onorm", bufs=6) as onormp,
            tc.tile_pool(name="ypool", bufs=4) as ypool,
            tc.tile_pool(name="dram", bufs=4, space="DRAM") as dram,
            tc.tile_pool(name="ps_s", bufs=2, space="PSUM") as ps_s,
            tc.tile_pool(name="ps_o", bufs=3, space="PSUM") as ps_o,
            tc.tile_pool(name="ps_fill", bufs=1, space="PSUM") as ps_fill,
        ):
            ident = consts.tile([128, 128], BF16)
            make_identity(nc, ident[:])
            onesb = consts.tile([128, 64], BF16)
            nc.vector.memset(onesb[:], 1.0)

            wq = weights.tile([128, CO, 128], BF16)
            wk = weights.tile([128, CO, 128], BF16)
            wv = weights.tile([128, CO, 128], BF16)
            nc.sync.dma_start(wq[:], wqT_ext.rearrange("(co p) d -> p co d", p=128))
            nc.sync.dma_start(wk[:], wkT_ext.rearrange("(co p) d -> p co d", p=128))
            nc.sync.dma_start(wv[:], wvT_ext.rearrange("(co p) d -> p co d", p=128))
            wp = weights.tile([128, C], BF16)
            nc.sync.dma_start(wp[:], wpT_ext.rearrange("h p o -> (h p) o"))

            from collections import deque

            filler = deque()

            def drain(n):
                for _ in range(n):
                    if filler:
                        filler.popleft()()

            def load_x_tile(b, t):
                """x^T columns for 512 tokens: 8 chunk tiles of [128, 512]."""
                xs = []
                for co in range(CO):
                    xc = xpool.tile([128, 512], BF16, tag="xchunk", name="xc")
                    nc.sync.dma_start(
                        xc[:],
                        xT_ext[co * 128:(co + 1) * 128,
                               b * N + t * 512:b * N + (t + 1) * 512],
                    )
                    xs.append(xc)
                return xs

            def qkv_chunk_units(b, t, xs, store):
                """One 512-token qkv tile -> QTc/KTc/VAc chunk tiles.

                Returns a list of single-PE-instruction closures."""
                units = []
                QTc = qkvp.tile([128, 512], BF16, tag="qt", name="qtc")
                KTc = qkvp.tile([128, 512], BF16, tag="kt", name="ktc")
                VAc = qkvp.tile([128, 4, 2 * (D + 1)], BF16, tag="vaug", name="vac")
                nc.vector.tensor_copy(VAc[:, :, D], onesb[:, 0:4])
                nc.vector.tensor_copy(VAc[:, :, 2 * D + 1], onesb[:, 0:4])
                store[t] = (QTc, KTc, VAc)

                def chain(w, writer):
                    hold = {}

                    def unit(co):
                        def f():
                            if co == 0:
                                hold["ps"] = ps_fill.tile(
                                    [128, 512], F32, tag="fill", name="fill_ps"
                                )
                            nc.tensor.matmul(
                                hold["ps"][:], lhsT=w[:, co, :], rhs=xs[co][:],
                                start=(co == 0), stop=(co == CO - 1),
                            )
                            if co == CO - 1:
                                writer(hold["ps"])
                        return f

                    return [unit(co) for co in range(CO)]

                def v_writer(ps):
                    vt = work.tile([128, 512], BF16, tag="vt", name="vt_stage")
                    nc.vector.tensor_copy(vt[:], ps[:])
                    for j in range(4):
                        vstg = work.tile(
                            [128, 128], BF16, tag="vstg", name="vstg"
                        )
                        nc.sync.dma_start_transpose(
                            vstg[:], vt[:, j * 128:(j + 1) * 128]
                        )
                        nc.vector.tensor_copy(
                            VAc[:, j, :]
                            .rearrange("p (h e) -> p h e", e=D + 1)[:, :, 0:D],
                            vstg.rearrange("p (h d) -> p h d", d=D),
                        )

                units += chain(wq, lambda ps: nc.vector.tensor_copy(QTc[:], ps[:]))
                units += chain(wk, lambda ps: nc.vector.tensor_copy(KTc[:], ps[:]))
                units += chain(wv, v_writer)
                return units

            def queue_proj(on_stk, b, qt):
                def unit(sq, ot):
                    def f():
                        sqs = slice(sq * 128, (sq + 1) * 128)
                        row0 = b * N + qt * 512 + sq * 128
                        os_ = slice(ot * 512, (ot + 1) * 512)
                        yps = ps_fill.tile(
                            [128, 512], F32, tag="fill", name="fill_y"
                        )
                        nc.tensor.matmul(
                            yps[:], lhsT=on_stk[:, sqs], rhs=wp[:, os_],
                            start=True, stop=True,
                        )
                        ysb = ypool.tile([128, 512], F32, tag="y", name="ysb")
                        nc.vector.tensor_copy(ysb[:], yps[:])
                        nc.sync.dma_start(out_ext[row0:row0 + 128, os_], ysb[:])
                    return f

                for sq in range(4):
                    for ot in range(2):
                        filler.append(unit(sq, ot))

            PVLAG = 3

            def norm_and_queue_proj(O0, O1, b, qt):
                on_stk = onormp.tile([128, 512], BF16, tag="onstk", name="onstk")
                for h, O in ((0, O0), (1, O1)):
                    rec = small.tile([128, 512], F32, tag="rec", name="rec")
                    nc.vector.reciprocal(rec[D:D + 1, :], O[D:D + 1, :])
                    dsc = dram.tile([1, 512], F32, tag="dscr", name="dscr")
                    nc.sync.dma_start(dsc[:], rec[D:D + 1, :])
                    bcs = small.tile([D, 512], F32, tag="bcs", name="bcs")
                    bsrc = bass.AP(
                        tensor=dsc.tensor, offset=dsc.offset,
                        ap=[[0, D]] + list(dsc.ap[1:]),
                    )
                    nc.sync.dma_start(bcs[:], bsrc)
                    if h == 0:
                        nc.vector.tensor_mul(on_stk[0:D, :], O[0:D, :], bcs[:])
                    else:
                        onh = onormp.tile([D, 512], BF16, tag="onh1", name="onh")
                        nc.vector.tensor_mul(onh[:], O[0:D, :], bcs[:])
                        nc.sync.dma_start(on_stk[D:2 * D, :], onh[:])
                queue_proj(on_stk, b, qt)

            # ---- schedule: one global software pipeline ----
            store0, store1 = {}, {}
            x0 = [load_x_tile(0, t) for t in range(NT)]
            u0 = [qkv_chunk_units(0, t, x0[t], store0) for t in range(NT)]
            for u in u0[0]:
                u()
            x1 = [load_x_tile(1, t) for t in range(NT)]
            u1 = [qkv_chunk_units(1, t, x1[t], store1) for t in range(NT)]

            # qkv work in 8-unit chain groups with stream deadlines
            qkv_groups = deque()
            for t in range(1, NT):
                for c in range(3):
                    qkv_groups.append((4 * t, u0[t][8 * c:8 * c + 8]))
            for t in range(NT):
                for c in range(3):
                    qkv_groups.append((64 + 4 * t, u1[t][8 * c:8 * c + 8]))
            n_groups_total = len(qkv_groups)
            pulled = [0]

            def pace_qkv(g):
                if qkv_groups and (
                    qkv_groups[0][0] <= g + 4
                    or pulled[0] < (g * n_groups_total) // 85
                ):
                    for u in qkv_groups.popleft()[1]:
                        u()
                    pulled[0] += 1

            stores = {0: store0, 1: store1}
            qts = [(0, q) for q in range(NT)] + [(1, q) for q in range(NT)]
            NQ = len(qts)
            Otiles = {}
            Ps = {}
            for g in range(NQ * NKT + PVLAG):
                # retire: PV pair for iteration g - PVLAG
                pv = g - PVLAG
                if pv >= 0:
                    bp, qtp = qts[pv // NKT]
                    ktp = pv % NKT
                    if ktp == 0:
                        Otiles[pv // NKT] = (
                            ps_o.tile([D + 1, 512], F32, tag="oacc", name="o0"),
                            ps_o.tile([D + 1, 512], F32, tag="oacc", name="o1"),
                        )
                    O0, O1 = Otiles[pv // NKT]
                    VAc = stores[bp][ktp // 4][2]
                    j = ktp % 4
                    P = Ps.pop(pv)
                    nc.tensor.matmul(
                        O0[:], lhsT=VAc[:, j, 0:D + 1], rhs=P[:, 0, :],
                        start=(ktp == 0), stop=(ktp == NKT - 1),
                    )
                    nc.tensor.matmul(
                        O1[:], lhsT=VAc[:, j, D + 1:2 * (D + 1)], rhs=P[:, 1, :],
                        start=(ktp == 0), stop=(ktp == NKT - 1),
                    )
                    if ktp == NKT - 1:
                        norm_and_queue_proj(O0, O1, bp, qtp)
                        del Otiles[pv // NKT]
                # issue: S pair + exp for iteration g
                if g < NQ * NKT:
                    b, qt = qts[g // NKT]
                    kt = g % NKT
                    pace_qkv(g)
                    QTc = stores[b][qt][0]
                    KTc = stores[b][kt // 4][1]
                    ks = slice((kt % 4) * 128, (kt % 4 + 1) * 128)
                    S = ps_s.tile([128, 2, 512], F32, tag="s", name="s")
                    nc.tensor.matmul(
                        S[:, 0, :], lhsT=KTc[0:D, ks], rhs=QTc[0:D, :],
                        start=True, stop=True, tile_position=(0, 0),
                    )
                    nc.tensor.matmul(
                        S[:, 1, :], lhsT=KTc[D:2 * D, ks], rhs=QTc[D:2 * D, :],
                        start=True, stop=True, tile_position=(64, 0),
                    )
                    P = work.tile([128, 2, 512], BF16, tag="p", name="p")
                    nc.scalar.activation(P[:], S[:], EXP)
                    Ps[g] = P
                # drip one proj filler per iteration
                if filler:
                    filler.popleft()()
            while filler or qkv_groups:
                if qkv_groups:
                    for u in qkv_groups.popleft()[1]:
                        u()
                elif filler:
                    filler.popleft()()
    nc.finalize()
    return nc


def _host_prep(x, W_qkv, W_proj):
    xT = np.ascontiguousarray(x.reshape(BN, C).T).astype(BFNP)
    in_maps = []
    for i in range(NCORES):
        hs = [HPC * i + j for j in range(HPC)]
        wq = np.concatenate([W_qkv[h * D:(h + 1) * D, :] for h in hs], 0)
        wk = np.concatenate([W_qkv[C + h * D:C + (h + 1) * D, :] for h in hs], 0)
        wv = np.concatenate([W_qkv[2 * C + h * D:2 * C + (h + 1) * D, :] for h in hs], 0)
        wqT = np.ascontiguousarray((wq * SCALE).T).astype(BFNP)
        wkT = np.ascontiguousarray(wk.T).astype(BFNP)
        wvT = np.ascontiguousarray(wv.T).astype(BFNP)
        wpT = np.stack(
            [np.ascontiguousarray(W_proj[:, h * D:(h + 1) * D].T) for h in hs], 0
        ).astype(BFNP)
        in_maps.append({"xT": xT, "wqT": wqT, "wkT": wkT, "wvT": wvT, "wpT": wpT})
    return in_maps


def run(x, W_qkv, W_proj, b_proj, trace=False):
    if "nc" not in _NC_CACHE:
        _NC_CACHE["nc"] = build_nc()
    nc = _NC_CACHE["nc"]
    in_maps = _host_prep(np.asarray(x), np.asarray(W_qkv), np.asarray(W_proj))
    res = run_bass_kernel_spmd(
        nc, in_maps, core_ids=list(range(NCORES)), trace=trace
    )
    acc = np.zeros((BN, C), np.float64)
    for i in range(NCORES):
        acc += res.results[i]["out"].astype(np.float64)
    y = (acc + np.asarray(b_proj).astype(np.float64)).astype(np.float32)
    return y.reshape(B, N, C), res


def kernel(x, W_qkv, W_proj, b_proj):
    y, _ = run(x, W_qkv, W_proj, b_proj, trace=False)
    return y


# revision 16
# speedup vs baseline: 1.0310x; 1.0310x over previous
"""Multi-head attention (B=2, N=2048, C=1024, H=16, D=64) on 8 TRN2 NeuronCores.

Sharding: tensor-parallel over heads. Core i owns heads (2i, 2i+1):
  - qkv weight columns for those heads (Q^T/K^T/V^T computed on device),
  - attention for 4 (batch, head) instances,
  - partial projection y_i = O_i @ W_proj[:, cols_i].T  (row-parallel proj).
Host gathers: y = sum_i y_i + b_proj.

Per-core pipeline (per batch):
  qkv:   Q^T,K^T [128(d,2 heads),2048] and V^T -> PE-transpose -> V_aug [n,130]
         (V columns + a ones column per head, so PV also yields softmax row-sums)
  attn:  per 512-query tile, loop over 16 key tiles:
         S^T[k,h,q] = K^T.T @ Q^T (bf16 in, f32 PSUM, the two heads run as
         packed row-group tiles), exp on ScalarE -> bf16,
         O~aug^T[65,q] += V_aug.T @ P~ (PSUM accumulation, row 64 = sum exp)
  norm:  recip(row 64) -> DMA partition-broadcast -> DVE mult
  proj:  y[q,o] = sum_h O_norm^T_h.T @ WpT_h (PSUM accumulation over heads)

Batch 1's qkv work is emitted interleaved with batch 0's attention so the
TensorEngine stays dense (HAM stays at full clock) while ScalarE runs exp.
Matmul operands are bf16; softmax statistics, PSUM accumulation and the
final output stay float32.
"""
import sys
import types

import numpy as np

B = 2
N = 2048
C = 1024
H = 16
D = 64
SCALE = D ** -0.5
NCORES = 8
HPC = H // NCORES  # heads per core = 2
BN = B * N


def _install_ntff_shim():
    """The image's antenv lacks axon_hooks; provide it so trace=True works."""
    if "antenv.axon_hooks" in sys.modules:
        return
    mod = types.ModuleType("antenv.axon_hooks")
    mod._HOOK = None
    mod.set_axon_ntff_profile_hook = lambda h: setattr(mod, "_HOOK", h)
    mod.get_axon_ntff_profile_hook = lambda: mod._HOOK
    sys.modules["antenv.axon_hooks"] = mod
    if "/root/.axon_site" not in sys.path:
        sys.path.insert(0, "/root/.axon_site")
    try:
        from trn_agent_boot.trn_boot import _ntff_profile_via_ctypes

        mod.set_axon_ntff_profile_hook(
            _ntff_profile_via_ctypes("/opt/axon/libaxon_pjrt.so")
        )
    except Exception:
        pass


_install_ntff_shim()

import ml_dtypes  # noqa: E402

import concourse.bass as bass  # noqa: E402
import concourse.tile as tile  # noqa: E402
from concourse import bacc, mybir  # noqa: E402
from concourse.bass_utils import run_bass_kernel_spmd  # noqa: E402
from concourse.masks import make_identity  # noqa: E402

F32 = mybir.dt.float32
BF16 = mybir.dt.bfloat16
EXP = mybir.ActivationFunctionType.Exp
BFNP = ml_dtypes.bfloat16

NT = N // 512          # 512-token tiles per batch (4)
NKT = N // 128         # 128-token key tiles per batch (16)
CO = C // 128          # contraction chunks (8)

_NC_CACHE = {}


def build_nc():
    nc = bacc.Bacc(None, target_bir_lowering=False)

    xT_ext = nc.declare_dram_parameter("xT", [C, BN], BF16, isOutput=False)
    wqT_ext = nc.declare_dram_parameter("wqT", [C, 128], BF16, isOutput=False)
    wkT_ext = nc.declare_dram_parameter("wkT", [C, 128], BF16, isOutput=False)
    wvT_ext = nc.declare_dram_parameter("wvT", [C, 128], BF16, isOutput=False)
    wpT_ext = nc.declare_dram_parameter("wpT", [HPC, D, C], BF16, isOutput=False)
    out_ext = nc.declare_dram_parameter("out", [BN, C], F32, isOutput=True)

    with tile.TileContext(nc) as tc:
        with (
            tc.tile_pool(name="consts", bufs=1) as consts,
            tc.tile_pool(name="weights", bufs=1) as weights,
            tc.tile_pool(name="xpool", bufs=8 * NT) as xpool,
            tc.tile_pool(name="qkvp", bufs=2 * NT) as qkvp,
            tc.tile_pool(name="work", bufs=8) as work,
            tc.tile_pool(name="small", bufs=3) as small,
            tc.tile_pool(name="onorm", bufs=6) as onormp,
            tc.tile_pool(name="ypool", bufs=4) as ypool,
            tc.tile_pool(name="dram", bufs=4, space="DRAM") as dram,
            tc.tile_pool(name="ps_s", bufs=2, space="PSUM") as ps_s,
            tc.tile_pool(name="ps_o", bufs=3, space="PSUM") as ps_o,
            tc.tile_pool(name="ps_fill", bufs=1, space="PSUM") as ps_fill,
        ):
            ident = consts.tile([128, 128], BF16)
            make_identity(nc, ident[:])
            onesb = consts.tile([128, 64], BF16)
            nc.vector.memset(onesb[:], 1.0)

            wq = weights.tile([128, CO, 128], BF16)
            wk = weights.tile([128, CO, 128], BF16)
            wv = weights.tile([128, CO, 128], BF16)
            nc.sync.dma_start(wq[:], wqT_ext.rearrange("(co p) d -> p co d", p=128))
            nc.sync.dma_start(wk[:], wkT_ext.rearrange("(co p) d -> p co d", p=128))
            nc.sync.dma_start(wv[:], wvT_ext.rearrange("(co p) d -> p co d", p=128))
            wp = weights.tile([128, C], BF16)
            nc.sync.dma_start(wp[:], wpT_ext.rearrange("h p o -> (h p) o"))

            from collections import deque

            filler = deque()

            def drain(n):
                for _ in range(n):
                    if filler:
                        filler.popleft()()

            def load_x_tile(b, t):
                """x^T columns for 512 tokens: 8 chunk tiles of [128, 512]."""
                xs = []
                for co in range(CO):
                    xc = xpool.tile([128, 512], BF16, tag="xchunk", name="xc")
                    nc.sync.dma_start(
                        xc[:],
                        xT_ext[co * 128:(co + 1) * 128,
                               b * N + t * 512:b * N + (t + 1) * 512],
                    )
                    xs.append(xc)
                return xs

            def qkv_chunk_units(b, t, xs, store):
                """One 512-token qkv tile -> QTc/KTc/VAc chunk tiles.

                Returns a list of single-PE-instruction closures."""
                units = []
                QTc = qkvp.tile([128, 512], BF16, tag="qt", name="qtc")
                KTc = qkvp.tile([128, 512], BF16, tag="kt", name="ktc")
                VAc = qkvp.tile([128, 4, 2 * (D + 1)], BF16, tag="vaug", name="vac")
                nc.vector.tensor_copy(VAc[:, :, D], onesb[:, 0:4])
                nc.vector.tensor_copy(VAc[:, :, 2 * D + 1], onesb[:, 0:4])
                store[t] = (QTc, KTc, VAc)

                def chain(w, writer):
                    hold = {}

                    def unit(co):
                        def f():
                            if co == 0:
                                hold["ps"] = ps_fill.tile(
                                    [128, 512], F32, tag="fill", name="fill_ps"
                                )
                            nc.tensor.matmul(
                                hold["ps"][:], lhsT=w[:, co, :], rhs=xs[co][:],
                                start=(co == 0), stop=(co == CO - 1),
                            )
                            if co == CO - 1:
                                writer(hold["ps"])
                        return f

                    return [unit(co) for co in range(CO)]

                def v_writer(ps):
                    vt = work.tile([128, 512], BF16, tag="vt", name="vt_stage")
                    nc.vector.tensor_copy(vt[:], ps[:])
                    for j in range(4):
                        vstg = work.tile(
                            [128, 128], BF16, tag="vstg", name="vstg"
                        )
                        nc.sync.dma_start_transpose(
                            vstg[:], vt[:, j * 128:(j + 1) * 128]
                        )
                        nc.vector.tensor_copy(
                            VAc[:, j, :]
                            .rearrange("p (h e) -> p h e", e=D + 1)[:, :, 0:D],
                            vstg.rearrange("p (h d) -> p h d", d=D),
                        )

                units += chain(wq, lambda ps: nc.vector.tensor_copy(QTc[:], ps[:]))
                units += chain(wk, lambda ps: nc.vector.tensor_copy(KTc[:], ps[:]))
                units += chain(wv, v_writer)
                return units

            def queue_proj(on_stk, b, qt):
                def unit(sq, ot):
                    def f():
                        sqs = slice(sq * 128, (sq + 1) * 128)
                        row0 = b * N + qt * 512 + sq * 128
                        os_ = slice(ot * 512, (ot + 1) * 512)
                        yps = ps_fill.tile(
                            [128, 512], F32, tag="fill", name="fill_y"
                        )
                        nc.tensor.matmul(
                            yps[:], lhsT=on_stk[:, sqs], rhs=wp[:, os_],
                            start=True, stop=True,
                        )
                        ysb = ypool.tile([128, 512], F32, tag="y", name="ysb")
                        nc.vector.tensor_copy(ysb[:], yps[:])
                        nc.sync.dma_start(out_ext[row0:row0 + 128, os_], ysb[:])
                    return f

                for sq in range(4):
                    for ot in range(2):
                        filler.append(unit(sq, ot))

            PVLAG = 3

            def norm_and_queue_proj(O0, O1, b, qt):
                on_stk = onormp.tile([128, 512], BF16, tag="onstk", name="onstk")
                for h, O in ((0, O0), (1, O1)):
                    rec = small.tile([128, 512], F32, tag="rec", name="rec")
                    nc.vector.reciprocal(rec[D:D + 1, :], O[D:D + 1, :])
                    dsc = dram.tile([1, 512], F32, tag="dscr", name="dscr")
                    nc.sync.dma_start(dsc[:], rec[D:D + 1, :])
                    bcs = small.tile([D, 512], F32, tag="bcs", name="bcs")
                    bsrc = bass.AP(
                        tensor=dsc.tensor, offset=dsc.offset,
                        ap=[[0, D]] + list(dsc.ap[1:]),
                    )
                    nc.sync.dma_start(bcs[:], bsrc)
                    if h == 0:
                        nc.vector.tensor_mul(on_stk[0:D, :], O[0:D, :], bcs[:])
                    else:
                        onh = onormp.tile([D, 512], BF16, tag="onh1", name="onh")
                        nc.vector.tensor_mul(onh[:], O[0:D, :], bcs[:])
                        nc.sync.dma_start(on_stk[D:2 * D, :], onh[:])
                queue_proj(on_stk, b, qt)

            # ---- schedule: one global software pipeline ----
            store0, store1 = {}, {}
            x0 = [load_x_tile(0, t) for t in range(NT)]
            u0 = [qkv_chunk_units(0, t, x0[t], store0) for t in range(NT)]
            for u in u0[0]:
                u()
            x1 = [load_x_tile(1, t) for t in range(NT)]
            u1 = [qkv_chunk_units(1, t, x1[t], store1) for t in range(NT)]

            # qkv work in full-chain groups (atomic: they share one PSUM slot)
            qkv_groups = deque()
            for t in range(1, NT):
                for c in range(3):
                    qkv_groups.append((4 * t, u0[t][8 * c:8 * c + 8]))
            for t in range(NT):
                for c in range(3):
                    qkv_groups.append((52 + 4 * t, u1[t][8 * c:8 * c + 8]))
            n_groups_total = len(qkv_groups)
            pulled = [0]

            def pace_qkv(g):
                if qkv_groups and (
                    qkv_groups[0][0] <= g + 3
                    or pulled[0] < (g * n_groups_total) // 70
                ):
                    for u in qkv_groups.popleft()[1]:
                        u()
                    pulled[0] += 1
                    return True
                return False

            stores = {0: store0, 1: store1}
            qts = [(0, q) for q in range(NT)] + [(1, q) for q in range(NT)]
            NQ = len(qts)
            Otiles = {}
            Ps = {}
            for g in range(NQ * NKT + PVLAG):
                # retire: PV pair for iteration g - PVLAG
                pv = g - PVLAG
                if pv >= 0:
                    bp, qtp = qts[pv // NKT]
                    ktp = pv % NKT
                    if ktp == 0:
                        Otiles[pv // NKT] = (
                            ps_o.tile([D + 1, 512], F32, tag="oacc", name="o0"),
                            ps_o.tile([D + 1, 512], F32, tag="oacc", name="o1"),
                        )
                    O0, O1 = Otiles[pv // NKT]
                    VAc = stores[bp][ktp // 4][2]
                    j = ktp % 4
                    P = Ps.pop(pv)
                    nc.tensor.matmul(
                        O0[:], lhsT=VAc[:, j, 0:D + 1], rhs=P[:, 0, :],
                        start=(ktp == 0), stop=(ktp == NKT - 1),
                    )
                    nc.tensor.matmul(
                        O1[:], lhsT=VAc[:, j, D + 1:2 * (D + 1)], rhs=P[:, 1, :],
                        start=(ktp == 0), stop=(ktp == NKT - 1),
                    )
                    if ktp == NKT - 1:
                        norm_and_queue_proj(O0, O1, bp, qtp)
                        del Otiles[pv // NKT]
                # issue: S pair + exp for iteration g
                chain_pulled = False
                if g < NQ * NKT:
                    b, qt = qts[g // NKT]
                    kt = g % NKT
                    chain_pulled = pace_qkv(g)
                    QTc = stores[b][qt][0]
                    KTc = stores[b][kt // 4][1]
                    ks = slice((kt % 4) * 128, (kt % 4 + 1) * 128)
                    S = ps_s.tile([128, 2, 512], F32, tag="s", name="s")
                    nc.tensor.matmul(
                        S[:, 0, :], lhsT=KTc[0:D, ks], rhs=QTc[0:D, :],
                        start=True, stop=True, tile_position=(0, 0),
                    )
                    nc.tensor.matmul(
                        S[:, 1, :], lhsT=KTc[D:2 * D, ks], rhs=QTc[D:2 * D, :],
                        start=True, stop=True, tile_position=(64, 0),
                    )
                    P = work.tile([128, 2, 512], BF16, tag="p", name="p")
                    nc.scalar.activation(P[:], S[:], EXP)
                    Ps[g] = P
                # drip one proj filler per iteration (not on chain iterations:
                # proj shares the single PSUM fill slot with open chains)
                if filler and not chain_pulled:
                    filler.popleft()()
            while filler or qkv_groups:
                if qkv_groups:
                    for u in qkv_groups.popleft()[1]:
                        u()
                elif filler:
                    filler.popleft()()
    nc.finalize()
    return nc


def _host_prep(x, W_qkv, W_proj):
    xT = np.ascontiguousarray(x.reshape(BN, C).T).astype(BFNP)
    in_maps = []
    for i in range(NCORES):
        hs = [HPC * i + j for j in range(HPC)]
        wq = np.concatenate([W_qkv[h * D:(h + 1) * D, :] for h in hs], 0)
        wk = np.concatenate([W_qkv[C + h * D:C + (h + 1) * D, :] for h in hs], 0)
        wv = np.concatenate([W_qkv[2 * C + h * D:2 * C + (h + 1) * D, :] for h in hs], 0)
        wqT = np.ascontiguousarray((wq * SCALE).T).astype(BFNP)
        wkT = np.ascontiguousarray(wk.T).astype(BFNP)
        wvT = np.ascontiguousarray(wv.T).astype(BFNP)
        wpT = np.stack(
            [np.ascontiguousarray(W_proj[:, h * D:(h + 1) * D].T) for h in hs], 0
        ).astype(BFNP)
        in_maps.append({"xT": xT, "wqT": wqT, "wkT": wkT, "wvT": wvT, "wpT": wpT})
    return in_maps


def run(x, W_qkv, W_proj, b_proj, trace=False):
    if "nc" not in _NC_CACHE:
        _NC_CACHE["nc"] = build_nc()
    nc = _NC_CACHE["nc"]
    in_maps = _host_prep(np.asarray(x), np.asarray(W_qkv), np.asarray(W_proj))
    res = run_bass_kernel_spmd(
        nc, in_maps, core_ids=list(range(NCORES)), trace=trace
    )
    acc = np.zeros((BN, C), np.float64)
    for i in range(NCORES):
        acc += res.results[i]["out"].astype(np.float64)
    y = (acc + np.asarray(b_proj).astype(np.float64)).astype(np.float32)
    return y.reshape(B, N, C), res


def kernel(x, W_qkv, W_proj, b_proj):
    y, _ = run(x, W_qkv, W_proj, b_proj, trace=False)
    return y
